# revision 3
# baseline (speedup 1.0000x reference)
"""Trainium2 Bass kernel for HDGradientCompressionLayer forward.

Reference computation: y = einsum("bsd,df->bsf", x, W) + b
  x: (4, 4096, 1024) f32, W: (1024, 1024) f32, b: (1024,) f32.

Strategy (data-parallel across 8 cores, per sharding hint):
  Flatten x to (16384, 1024); each core gets 2048 rows (= 16 rowblocks
  of 128).  All layout work happens on the HOST so the device does pure
  HWDGE copy DMAs and the PE does only the 256 bf16 matmuls:
    - host casts x/W to bf16 and pre-transposes each core's x shard to
      xT [d, m]; W and the first 512 xT columns are packed into "wxa"
      [1024, 1536] so a single [128, 1536] DMA per d-block delivers both
      the W k-tile and the x columns the first rowblock group needs,
    - device: rowblock groups of (4,4,4,3,1) (PSUM-bank limited),
      k-outer accumulation psum[m,f] += xT[k][:,m-slice].T @ W[k][:,f],
      the 1-rowblock final group keeps the post-matmul tail short,
    - DVE adds the (partition-broadcast) f32 bias during PSUM->SBUF
      eviction, scalar(ACT) HWDGE stores the f32 y rowblock.
"""

import os

import numpy as np

import concourse.bass as bass
import concourse.bacc as bacc
import concourse.tile as tile
from concourse import mybir
from concourse.bass_utils import run_bass_kernel_spmd

N_CORES = 8
B, S, D = 4, 4096, 1024
F = 1024
ROWS_TOTAL = B * S            # 16384
ROWS = ROWS_TOTAL // N_CORES  # 2048 per core
P = 128
NSPLIT = 512                  # one PSUM bank of f32
KB = D // P                   # 8 contraction blocks
RB = ROWS // P                # 16 rowblocks per core
GROUPS = (4, 4, 4, 3, 1)      # rowblocks per PSUM group (<=4: 8 banks)
MA = 512                      # xT columns packed with W into wxa
MB = 512                      # xT columns in wxb
MR = ROWS - MA - MB           # 1024 xT columns in xr

_BF16 = mybir.dt.np(mybir.dt.bfloat16)


def build_nc() -> bass.Bass:
    nc = bacc.Bacc("TRN2", target_bir_lowering=False, debug=False)
    # wxa[d, 0:F] = W[d, :] (bf16);  wxa[d, F:] = xT[d, 0:MA]
    wxa = nc.dram_tensor("wxa", [D, F + MA], mybir.dt.bfloat16, kind="ExternalInput").ap()
    wxb = nc.dram_tensor("wxb", [D, MB], mybir.dt.bfloat16, kind="ExternalInput").ap()
    xr = nc.dram_tensor("xr", [D, MR], mybir.dt.bfloat16, kind="ExternalInput").ap()
    b = nc.dram_tensor("b", [F], mybir.dt.float32, kind="ExternalInput").ap()
    y = nc.dram_tensor("y", [ROWS, F], mybir.dt.float32, kind="ExternalOutput").ap()

    with tile.TileContext(nc) as tc:
        with tc.tile_pool(name="const", bufs=1) as const, \
             tc.tile_pool(name="ap", bufs=1) as apool, \
             tc.tile_pool(name="bp", bufs=1) as bpool, \
             tc.tile_pool(name="rp", bufs=1) as rpool, \
             tc.tile_pool(name="yp", bufs=1) as yp, \
             tc.tile_pool(name="psp", bufs=1, space="PSUM") as psp:

            # HAM warmup: dependency-free 1-partition matmuls keep the PE
            # activity window hot from the moment the body starts, so the
            # clock is at 8/8 when the first data-carrying matmul issues.
            warm = const.tile([1, NSPLIT], mybir.dt.bfloat16)
            nc.vector.memset(warm[:], 0.0)
            warm_ps = psp.tile([P, NSPLIT], mybir.dt.float32, tag="ps_0_0", bufs=1)
            for _ in range(6):
                nc.tensor.matmul(warm_ps[0:1, :], warm[0:1, 0:1], warm[:],
                                 start=True, stop=True, skip_group_check=True)

            # Bias broadcast to all partitions, f32 (SWDGE broadcast, tiny).
            b_bc = const.tile([P, F], mybir.dt.float32)
            nc.gpsimd.dma_start(b_bc[:], b.rearrange("(o f) -> o f", o=1).to_broadcast([P, F]))

            # Loads on the SP HWDGE ring, in consumption order.
            wxa_t, wxb_t, xr_t = [], [], []
            for k in range(KB):
                t = apool.tile([P, F + MA], mybir.dt.bfloat16, name=f"wxa{k}", tag=f"wxa{k}")
                nc.sync.dma_start(t[:], wxa[k * P:(k + 1) * P, :])
                wxa_t.append(t)
            for k in range(KB):
                t = bpool.tile([P, MB], mybir.dt.bfloat16, name=f"wxb{k}", tag=f"wxb{k}")
                nc.sync.dma_start(t[:], wxb[k * P:(k + 1) * P, :])
                wxb_t.append(t)
            for k in range(KB):
                t = rpool.tile([P, MR], mybir.dt.bfloat16, name=f"xr{k}", tag=f"xr{k}")
                nc.sync.dma_start(t[:], xr[k * P:(k + 1) * P, :])
                xr_t.append(t)

            def xslice(k: int, rb: int):
                m0 = rb * P
                if m0 < MA:
                    return wxa_t[k][:, F + m0:F + m0 + P]
                if m0 < MA + MB:
                    return wxb_t[k][:, m0 - MA:m0 - MA + P]
                return xr_t[k][:, m0 - MA - MB:m0 - MA - MB + P]

            rb0 = 0
            for gi, gsz in enumerate(GROUPS):
                # The 1-rowblock final group takes the ps_3 tags (last
                # evicted two groups ago) so its k=0 matmul never waits.
                joff = 3 if gsz == 1 else 0
                ps = [[psp.tile([P, NSPLIT], mybir.dt.float32, name=f"ps_{j + joff}_{n}",
                                tag=f"ps_{j + joff}_{n}", bufs=1)
                       for n in range(2)] for j in range(gsz)]
                for k in range(KB):
                    for j in range(gsz):
                        xs = xslice(k, rb0 + j)
                        for n in range(2):
                            nc.tensor.matmul(
                                ps[j][n][:],
                                xs,
                                wxa_t[k][:, n * NSPLIT:(n + 1) * NSPLIT],
                                start=(k == 0),
                                stop=(k == KB - 1),
                            )
                for j in range(gsz):
                    rb = rb0 + j
                    last = rb == RB - 1
                    y_sb = yp.tile([P, F], mybir.dt.float32, name="ysb", tag="ysb", bufs=6)
                    for n in range(2):
                        nc.vector.tensor_add(
                            y_sb[:, n * NSPLIT:(n + 1) * NSPLIT],
                            ps[j][n][:],
                            b_bc[:, n * NSPLIT:(n + 1) * NSPLIT],
                        )
                        if last:
                            # Split the final store so each half chases its
                            # eviction — shortens the post-matmul tail.
                            nc.scalar.dma_start(
                                y[rb * P:(rb + 1) * P, n * NSPLIT:(n + 1) * NSPLIT],
                                y_sb[:, n * NSPLIT:(n + 1) * NSPLIT],
                            )
                    if not last:
                        nc.scalar.dma_start(y[rb * P:(rb + 1) * P, :], y_sb[:])
                rb0 += gsz

    nc.compile()
    return nc


_NC_CACHE: dict[str, bass.Bass] = {}


def _get_nc() -> bass.Bass:
    if "nc" not in _NC_CACHE:
        _NC_CACHE["nc"] = build_nc()
    return _NC_CACHE["nc"]


def _run(in_maps, trace: bool = False):
    nc = _get_nc()
    return run_bass_kernel_spmd(nc, in_maps, list(range(N_CORES)), trace=trace)


def make_in_maps(x: np.ndarray, W: np.ndarray, b: np.ndarray):
    x = np.ascontiguousarray(np.asarray(x, dtype=np.float32)).reshape(ROWS_TOTAL, D)
    W_bf = np.asarray(W, dtype=np.float32).astype(_BF16)
    b = np.ascontiguousarray(np.asarray(b, dtype=np.float32))
    in_maps = []
    for c in range(N_CORES):
        shard_bf = x[c * ROWS:(c + 1) * ROWS].astype(_BF16)  # [2048, 1024]
        wxa = np.empty((D, F + MA), dtype=_BF16)
        wxa[:, :F] = W_bf
        wxa[:, F:] = shard_bf[:MA].T
        wxb = np.ascontiguousarray(shard_bf[MA:MA + MB].T)
        xr = np.ascontiguousarray(shard_bf[MA + MB:].T)
        in_maps.append({"wxa": wxa, "wxb": wxb, "xr": xr, "b": b})
    return in_maps


def kernel(x: np.ndarray, W: np.ndarray, b: np.ndarray) -> np.ndarray:
    in_maps = make_in_maps(x, W, b)
    res = _run(in_maps, trace=bool(int(os.environ.get("BASS_KERNEL_TRACE", "0"))))
    y = np.concatenate([res.results[c]["y"] for c in range(N_CORES)], axis=0)
    return y.reshape(B, S, F)


# revision 5
# speedup vs baseline: 1.1986x; 1.1986x over previous
"""Trainium2 Bass kernel for HDGradientCompressionLayer forward.

Reference computation: y = einsum("bsd,df->bsf", x, W) + b
  x: (4, 4096, 1024) f32, W: (1024, 1024) f32, b: (1024,) f32.

Strategy (data-parallel across 8 cores, per sharding hint):
  Flatten x to (16384, 1024); each core gets 2048 rows (= 16 rowblocks
  of 128).  All layout work happens on the HOST so the device does pure
  HWDGE copy DMAs and the PE does only the 256 bf16 matmuls:
    - host casts x/W to bf16 and pre-transposes each core's x shard to
      xT [d, m]; W and the first 512 xT columns are packed into "wxa"
      [1024, 1536] so a single [128, 1536] DMA per d-block delivers both
      the W k-tile and the x columns the first rowblock group needs,
    - device: rowblock groups of (4,4,4,3,1) (PSUM-bank limited),
      k-outer accumulation psum[m,f] += xT[k][:,m-slice].T @ W[k][:,f],
      the 1-rowblock final group keeps the post-matmul tail short,
    - DVE adds the (partition-broadcast) f32 bias during PSUM->SBUF
      eviction, scalar(ACT) HWDGE stores the f32 y rowblock.
"""

import os

import numpy as np

import concourse.bass as bass
import concourse.bacc as bacc
import concourse.tile as tile
from concourse import mybir
from concourse.bass_utils import run_bass_kernel_spmd

N_CORES = 8
B, S, D = 4, 4096, 1024
F = 1024
ROWS_TOTAL = B * S            # 16384
ROWS = ROWS_TOTAL // N_CORES  # 2048 per core
P = 128
NSPLIT = 512                  # one PSUM bank of f32
KB = D // P                   # 8 contraction blocks
RB = ROWS // P                # 16 rowblocks per core
GROUPS = (4, 4, 4, 3, 1)      # rowblocks per PSUM group (<=4: 8 banks)
MA = 512                      # xT columns packed with W into wxa
MB = 512                      # xT columns in wxb
MR = ROWS - MA - MB           # 1024 xT columns in xr

_BF16 = mybir.dt.np(mybir.dt.bfloat16)


def build_nc() -> bass.Bass:
    nc = bacc.Bacc("TRN2", target_bir_lowering=False, debug=False)
    # wxa[d, 0:F] = W[d, :] (bf16);  wxa[d, F:] = xT[d, 0:MA]
    wxa = nc.dram_tensor("wxa", [D, F + MA], mybir.dt.bfloat16, kind="ExternalInput").ap()
    wxb = nc.dram_tensor("wxb", [D, MB], mybir.dt.bfloat16, kind="ExternalInput").ap()
    xr = nc.dram_tensor("xr", [D, MR], mybir.dt.bfloat16, kind="ExternalInput").ap()
    b = nc.dram_tensor("b", [F], mybir.dt.float32, kind="ExternalInput").ap()
    y = nc.dram_tensor("y", [ROWS, F], mybir.dt.float32, kind="ExternalOutput").ap()

    with tile.TileContext(nc) as tc:
        with tc.tile_pool(name="const", bufs=1) as const, \
             tc.tile_pool(name="ap", bufs=1) as apool, \
             tc.tile_pool(name="bp", bufs=1) as bpool, \
             tc.tile_pool(name="rp", bufs=1) as rpool, \
             tc.tile_pool(name="yp", bufs=1) as yp, \
             tc.tile_pool(name="psp", bufs=1, space="PSUM") as psp:

            # HAM warmup: full-array matmuls on a zeroed tile (1-partition
            # ones do NOT register as PE activity) so the clock is at 8/8
            # when the first data-carrying matmul issues.
            warm = const.tile([P, NSPLIT], mybir.dt.bfloat16)
            nc.vector.memset(warm[:], 0.0)
            warm_ps = psp.tile([P, NSPLIT], mybir.dt.float32, tag="ps_0_0", bufs=1)
            for _ in range(5):
                nc.tensor.matmul(warm_ps[:], warm[:, 0:P], warm[:],
                                 start=True, stop=True, skip_group_check=True)

            # Bias broadcast to all partitions, f32 (SWDGE broadcast, tiny).
            b_bc = const.tile([P, F], mybir.dt.float32)
            nc.gpsimd.dma_start(b_bc[:], b.rearrange("(o f) -> o f", o=1).to_broadcast([P, F]))

            # Loads on the SP HWDGE ring, in consumption order.
            wxa_t, wxb_t, xr_t = [], [], []
            for k in range(KB):
                t = apool.tile([P, F + MA], mybir.dt.bfloat16, name=f"wxa{k}", tag=f"wxa{k}")
                if k == 0:
                    # Split so the first group's j=0 matmuls (needing only
                    # W k0 + the first 128 xT cols) start one DMA earlier.
                    nc.sync.dma_start(t[:, :F + P], wxa[0:P, :F + P])
                    nc.sync.dma_start(t[:, F + P:], wxa[0:P, F + P:])
                else:
                    nc.sync.dma_start(t[:], wxa[k * P:(k + 1) * P, :])
                wxa_t.append(t)
            for k in range(KB):
                t = bpool.tile([P, MB], mybir.dt.bfloat16, name=f"wxb{k}", tag=f"wxb{k}")
                nc.sync.dma_start(t[:], wxb[k * P:(k + 1) * P, :])
                wxb_t.append(t)
            for k in range(KB):
                t = rpool.tile([P, MR], mybir.dt.bfloat16, name=f"xr{k}", tag=f"xr{k}")
                nc.sync.dma_start(t[:], xr[k * P:(k + 1) * P, :])
                xr_t.append(t)

            def xslice(k: int, rb: int):
                m0 = rb * P
                if m0 < MA:
                    return wxa_t[k][:, F + m0:F + m0 + P]
                if m0 < MA + MB:
                    return wxb_t[k][:, m0 - MA:m0 - MA + P]
                return xr_t[k][:, m0 - MA - MB:m0 - MA - MB + P]

            rb0 = 0
            for gi, gsz in enumerate(GROUPS):
                # The 1-rowblock final group takes the ps_3 tags (last
                # evicted two groups ago) so its k=0 matmul never waits.
                joff = 3 if gsz == 1 else 0
                ps = [[psp.tile([P, NSPLIT], mybir.dt.float32, name=f"ps_{j + joff}_{n}",
                                tag=f"ps_{j + joff}_{n}", bufs=1)
                       for n in range(2)] for j in range(gsz)]
                for k in range(KB):
                    for j in range(gsz):
                        xs = xslice(k, rb0 + j)
                        for n in range(2):
                            nc.tensor.matmul(
                                ps[j][n][:],
                                xs,
                                wxa_t[k][:, n * NSPLIT:(n + 1) * NSPLIT],
                                start=(k == 0),
                                stop=(k == KB - 1),
                            )
                for j in range(gsz):
                    rb = rb0 + j
                    last = rb == RB - 1
                    y_sb = yp.tile([P, F], mybir.dt.float32, name="ysb", tag="ysb", bufs=6)
                    for n in range(2):
                        nc.vector.tensor_add(
                            y_sb[:, n * NSPLIT:(n + 1) * NSPLIT],
                            ps[j][n][:],
                            b_bc[:, n * NSPLIT:(n + 1) * NSPLIT],
                        )
                        if last:
                            # Split the final store so each half chases its
                            # eviction — shortens the post-matmul tail.
                            nc.scalar.dma_start(
                                y[rb * P:(rb + 1) * P, n * NSPLIT:(n + 1) * NSPLIT],
                                y_sb[:, n * NSPLIT:(n + 1) * NSPLIT],
                            )
                    if not last:
                        nc.scalar.dma_start(y[rb * P:(rb + 1) * P, :], y_sb[:])
                rb0 += gsz

    nc.compile()
    return nc


_NC_CACHE: dict[str, bass.Bass] = {}


def _get_nc() -> bass.Bass:
    if "nc" not in _NC_CACHE:
        _NC_CACHE["nc"] = build_nc()
    return _NC_CACHE["nc"]


def _run(in_maps, trace: bool = False):
    nc = _get_nc()
    return run_bass_kernel_spmd(nc, in_maps, list(range(N_CORES)), trace=trace)


def make_in_maps(x: np.ndarray, W: np.ndarray, b: np.ndarray):
    x = np.ascontiguousarray(np.asarray(x, dtype=np.float32)).reshape(ROWS_TOTAL, D)
    W_bf = np.asarray(W, dtype=np.float32).astype(_BF16)
    b = np.ascontiguousarray(np.asarray(b, dtype=np.float32))
    in_maps = []
    for c in range(N_CORES):
        shard_bf = x[c * ROWS:(c + 1) * ROWS].astype(_BF16)  # [2048, 1024]
        wxa = np.empty((D, F + MA), dtype=_BF16)
        wxa[:, :F] = W_bf
        wxa[:, F:] = shard_bf[:MA].T
        wxb = np.ascontiguousarray(shard_bf[MA:MA + MB].T)
        xr = np.ascontiguousarray(shard_bf[MA + MB:].T)
        in_maps.append({"wxa": wxa, "wxb": wxb, "xr": xr, "b": b})
    return in_maps


def kernel(x: np.ndarray, W: np.ndarray, b: np.ndarray) -> np.ndarray:
    in_maps = make_in_maps(x, W, b)
    res = _run(in_maps, trace=bool(int(os.environ.get("BASS_KERNEL_TRACE", "0"))))
    y = np.concatenate([res.results[c]["y"] for c in range(N_CORES)], axis=0)
    return y.reshape(B, S, F)
